# revision 7
# baseline (speedup 1.0000x reference)
"""Trainium2 Bass kernel for CustomFullyConnectedLayerGoogleTopK.

Math (from the reference, with IN_F == OUT_F == TOTAL_PERM == DIAG_LEN == 4096):
    a_topk = clip(K * softmax(alpha), 0, 1)                    # K = 3687
    Vs     = V * a_topk[:, None]                               # [4096, 4096]
    W[r,c] = Vs[(r - c) % 4096, c]   (scatter has no collisions)
    out    = x @ W.T                                           # [8192, 4096]

Device strategy: data-parallel over batch (8 cores x 1024 rows). The weight
W.T[c, r] = VsT[c, (r - c) % 4096] where VsT = Vs.T. Storing the doubled
array W2 = concat(VsT, VsT, axis=1) [4096, 8192] makes every [128, ntile]
tile of W.T a single affine access pattern: element (p, j) of the tile for
(k, n) lives at W2 linear offset (4096 + n*ntile) + p*8191 + k*128*8191 + j.
So the whole matmul streams with plain DMAs - no gather, no transpose.

Each core: out_slice[1024, 4096] = xT_slice.T @ W.T via PE matmuls,
lhsT = xT tile [128c, 128b] (stationary), rhs = W.T tile [128c, ntile r].
"""

import os

import numpy as np
import ml_dtypes

B = 8192  # batch
F = 4096  # in_features == out_features == total_perm == diag_len
NCORES = 8
BS = B // NCORES  # batch rows per core
KTOPK = 3687  # ceil((1 - 0.1) * F * F / F)

# "bf16" (full-rate PE, ~1.6e-3 rel err) or "fp32r" (fp32 storage,
# reduced-precision multiply).
_MODE = os.environ.get("GTOPK_MODE", "bf16")

_NC_CACHE = {}
_LAST_RESULTS = None  # stashed BassKernelResults for test harness introspection


def _build_nc(mode, f=F, bs=BS):
    import concourse.bass as bass
    import concourse.tile as tile
    from concourse import bacc, mybir

    if mode == "bf16":
        in_dt = mybir.dt.bfloat16
        n_tile = 512
    elif mode == "fp32r":
        in_dt = mybir.dt.float32r
        n_tile = 256
    else:
        raise ValueError(mode)

    k_tiles = f // 128
    m_tiles = bs // 128
    n_tiles = f // n_tile
    w2w = 2 * f  # doubled width

    nc = bacc.Bacc(None, target_bir_lowering=False, debug=False)
    xt = nc.dram_tensor("xt", [f, bs], in_dt, kind="ExternalInput")
    w2 = nc.dram_tensor("w2", [f, w2w], in_dt, kind="ExternalInput")
    out = nc.dram_tensor("out", [bs, f], mybir.dt.float32, kind="ExternalOutput")

    def xt_src(k):  # [128, bs] tile k of x.T slice
        return bass.AP(xt, k * 128 * bs, [[bs, 128], [1, bs]])

    def wt_src(n, k):  # staircase [128, n_tile] tile of W.T
        return bass.AP(
            w2, f + n * n_tile + k * 128 * (w2w - 1), [[w2w - 1, 128], [1, n_tile]]
        )

    with tile.TileContext(nc) as tc:
        with (
            tc.tile_pool(name="xpool", bufs=k_tiles) as xpool,
            tc.tile_pool(name="wpool", bufs=2 * k_tiles) as wpool,
            tc.tile_pool(name="opool", bufs=4) as opool,
            tc.tile_pool(name="ppool", bufs=8, space="PSUM") as ppool,
        ):
            # x.T slice cached in SBUF as 32 separate [128, bs] tiles so the
            # scheduler can start matmuls as soon as individual tiles land.
            # Interleave x/w DMAs for n=0 so (xt[k], wt[0,k]) pairs arrive
            # together.
            xts = []
            wt0 = []
            for k in range(k_tiles):
                xk = xpool.tile([128, bs], in_dt, name=f"xt{k}", tag="xt")
                nc.sync.dma_start(out=xk[:], in_=xt_src(k))
                wk = wpool.tile([128, n_tile], in_dt, name=f"wt0_{k}", tag="wt")
                nc.sync.dma_start(out=wk[:], in_=wt_src(0, k))
                xts.append(xk)
                wt0.append(wk)

            wts = wt0
            for n in range(n_tiles):
                # prefetch next n's weight tiles
                if n + 1 < n_tiles:
                    nxt = []
                    for k in range(k_tiles):
                        wk = wpool.tile(
                            [128, n_tile], in_dt, name=f"wt{n + 1}_{k}", tag="wt"
                        )
                        nc.sync.dma_start(out=wk[:], in_=wt_src(n + 1, k))
                        nxt.append(wk)
                if n == 0:
                    # Ramp phase: k-outer / m-inner over the first half of k
                    # so each arriving (xt[k], wt[k]) pair immediately feeds
                    # m_tiles matmuls (PE starts as soon as the first pair
                    # lands). Then finish per-m (k-inner) so the 8 psum banks
                    # complete staggered and evictions overlap compute.
                    k_half = k_tiles // 2
                    pss = [
                        ppool.tile([128, n_tile], mybir.dt.float32, name=f"ps{m}", tag="ps")
                        for m in range(m_tiles)
                    ]
                    for k in range(k_half):
                        for m in range(m_tiles):
                            nc.tensor.matmul(
                                pss[m][:],
                                xts[k][:, m * 128 : (m + 1) * 128],
                                wts[k][:],
                                start=(k == 0),
                                stop=False,
                                skip_group_check=True,
                            )
                    for m in range(m_tiles):
                        for k in range(k_half, k_tiles):
                            nc.tensor.matmul(
                                pss[m][:],
                                xts[k][:, m * 128 : (m + 1) * 128],
                                wts[k][:],
                                start=False,
                                stop=(k == k_tiles - 1),
                                skip_group_check=True,
                            )
                        o_sb = opool.tile([128, n_tile], mybir.dt.float32, name="o_sb", tag="o_sb")
                        nc.vector.tensor_copy(o_sb[:], pss[m][:])
                        nc.scalar.dma_start(
                            out=bass.AP(
                                out, m * 128 * f + n * n_tile, [[f, 128], [1, n_tile]]
                            ),
                            in_=o_sb[:],
                        )
                else:
                    # m-outer / k-inner: staggered psum completion overlaps
                    # eviction + output DMA with compute.
                    for m in range(m_tiles):
                        ps = ppool.tile([128, n_tile], mybir.dt.float32, name="ps", tag="ps")
                        for k in range(k_tiles):
                            nc.tensor.matmul(
                                ps[:],
                                xts[k][:, m * 128 : (m + 1) * 128],
                                wts[k][:],
                                start=(k == 0),
                                stop=(k == k_tiles - 1),
                            )
                        o_sb = opool.tile([128, n_tile], mybir.dt.float32, name="o_sb", tag="o_sb")
                        nc.vector.tensor_copy(o_sb[:], ps[:])
                        nc.scalar.dma_start(
                            out=bass.AP(
                                out, m * 128 * f + n * n_tile, [[f, 128], [1, n_tile]]
                            ),
                            in_=o_sb[:],
                        )
                if n + 1 < n_tiles:
                    wts = nxt
    nc.compile()
    return nc


def _get_nc(mode):
    if mode not in _NC_CACHE:
        _NC_CACHE[mode] = _build_nc(mode)
    return _NC_CACHE[mode]


def _soft_topk_scale(alpha):
    a = alpha.astype(np.float64)
    e = np.exp(a - a.max())
    return np.clip(KTOPK * (e / e.sum()), 0.0, 1.0).astype(np.float32)


def kernel(x, V, alpha):
    global _LAST_RESULTS
    from concourse.bass_utils import run_bass_kernel_spmd

    x = np.asarray(x, dtype=np.float32)
    V = np.asarray(V, dtype=np.float32)
    alpha = np.asarray(alpha, dtype=np.float32)

    a_topk = _soft_topk_scale(alpha)
    VsT = np.ascontiguousarray((V * a_topk[:, None]).T)  # [c, p]
    W2 = np.concatenate([VsT, VsT], axis=1)  # [F, 2F]
    xT = np.ascontiguousarray(x.T)  # [F, B]

    mode = _MODE
    if mode == "bf16":
        W2 = W2.astype(ml_dtypes.bfloat16)
        xT = xT.astype(ml_dtypes.bfloat16)

    nc = _get_nc(mode)
    in_maps = [
        {"xt": np.ascontiguousarray(xT[:, i * BS : (i + 1) * BS]), "w2": W2}
        for i in range(NCORES)
    ]
    kwargs = {}
    if os.environ.get("GTOPK_TRACE"):
        kwargs["trace"] = True
    res = run_bass_kernel_spmd(nc, in_maps, core_ids=list(range(NCORES)), **kwargs)
    _LAST_RESULTS = res
    return np.concatenate([r["out"] for r in res.results], axis=0)


# revision 8
# speedup vs baseline: 1.1861x; 1.1861x over previous
"""Trainium2 Bass kernel for CustomFullyConnectedLayerGoogleTopK.

Math (from the reference, with IN_F == OUT_F == TOTAL_PERM == DIAG_LEN == 4096):
    a_topk = clip(K * softmax(alpha), 0, 1)                    # K = 3687
    Vs     = V * a_topk[:, None]                               # [4096, 4096]
    W[r,c] = Vs[(r - c) % 4096, c]   (scatter has no collisions)
    out    = x @ W.T                                           # [8192, 4096]

Device strategy: data-parallel over batch (8 cores x 1024 rows). The weight
W.T[c, r] = VsT[c, (r - c) % 4096] where VsT = Vs.T. Storing the doubled
array W2 = concat(VsT, VsT, axis=1) [4096, 8192] makes every [128, ntile]
tile of W.T a single affine access pattern: element (p, j) of the tile for
(k, n) lives at W2 linear offset (4096 + n*ntile) + p*8191 + k*128*8191 + j.
So the whole matmul streams with plain DMAs - no gather, no transpose.

Each core: out_slice[1024, 4096] = xT_slice.T @ W.T via PE matmuls,
lhsT = xT tile [128c, 128b] (stationary), rhs = W.T tile [128c, ntile r].
"""

import os

import numpy as np
import ml_dtypes

B = 8192  # batch
F = 4096  # in_features == out_features == total_perm == diag_len
NCORES = 8
BS = B // NCORES  # batch rows per core
KTOPK = 3687  # ceil((1 - 0.1) * F * F / F)

# "bf16" (full-rate PE, ~1.6e-3 rel err) or "fp32r" (fp32 storage,
# reduced-precision multiply).
_MODE = os.environ.get("GTOPK_MODE", "bf16")

_NC_CACHE = {}
_LAST_RESULTS = None  # stashed BassKernelResults for test harness introspection


def _build_nc(mode, f=F, bs=BS):
    import concourse.bass as bass
    import concourse.tile as tile
    from concourse import bacc, mybir

    if mode == "bf16":
        in_dt = mybir.dt.bfloat16
        n_tile = 512
    elif mode == "fp32r":
        in_dt = mybir.dt.float32r
        n_tile = 256
    else:
        raise ValueError(mode)

    k_tiles = f // 128
    m_tiles = bs // 128
    n_tiles = f // n_tile
    w2w = 2 * f  # doubled width

    nc = bacc.Bacc(None, target_bir_lowering=False, debug=False)
    xt = nc.dram_tensor("xt", [f, bs], in_dt, kind="ExternalInput")
    w2 = nc.dram_tensor("w2", [f, w2w], in_dt, kind="ExternalInput")
    out = nc.dram_tensor("out", [bs, f], mybir.dt.float32, kind="ExternalOutput")

    def xt_src(k):  # [128, bs] tile k of x.T slice
        return bass.AP(xt, k * 128 * bs, [[bs, 128], [1, bs]])

    def wt_src(n, k):  # staircase [128, n_tile] tile of W.T
        return bass.AP(
            w2, f + n * n_tile + k * 128 * (w2w - 1), [[w2w - 1, 128], [1, n_tile]]
        )

    with tile.TileContext(nc) as tc:
        with (
            tc.tile_pool(name="xpool", bufs=k_tiles) as xpool,
            tc.tile_pool(name="wpool", bufs=2 * k_tiles) as wpool,
            tc.tile_pool(name="opool", bufs=4) as opool,
            tc.tile_pool(name="ppool", bufs=8, space="PSUM") as ppool,
        ):
            # x.T slice cached in SBUF as 32 separate [128, bs] tiles so the
            # scheduler can start matmuls as soon as individual tiles land.
            # Interleave x/w DMAs for n=0 so (xt[k], wt[0,k]) pairs arrive
            # together.
            # xt rides the ACT HWDGE ring, wt the SP ring: the two issue
            # sequencers run in parallel, so the first (xt[0], wt[0,0]) pair
            # is in flight after one ~0.6us DMA issue instead of two.
            xts = []
            wt0 = []
            for k in range(k_tiles):
                wk = wpool.tile([128, n_tile], in_dt, name=f"wt0_{k}", tag="wt")
                nc.sync.dma_start(out=wk[:], in_=wt_src(0, k))
                xk = xpool.tile([128, bs], in_dt, name=f"xt{k}", tag="xt")
                nc.scalar.dma_start(out=xk[:], in_=xt_src(k))
                xts.append(xk)
                wt0.append(wk)

            wts = wt0
            for n in range(n_tiles):
                # prefetch next n's weight tiles
                if n + 1 < n_tiles:
                    nxt = []
                    for k in range(k_tiles):
                        wk = wpool.tile(
                            [128, n_tile], in_dt, name=f"wt{n + 1}_{k}", tag="wt"
                        )
                        nc.sync.dma_start(out=wk[:], in_=wt_src(n + 1, k))
                        nxt.append(wk)
                if n == 0:
                    # Ramp phase: k-outer / m-inner over the first half of k
                    # so each arriving (xt[k], wt[k]) pair immediately feeds
                    # m_tiles matmuls (PE starts as soon as the first pair
                    # lands). Then finish per-m (k-inner) so the 8 psum banks
                    # complete staggered and evictions overlap compute.
                    k_half = k_tiles // 2
                    pss = [
                        ppool.tile([128, n_tile], mybir.dt.float32, name=f"ps{m}", tag="ps")
                        for m in range(m_tiles)
                    ]
                    for k in range(k_half):
                        for m in range(m_tiles):
                            nc.tensor.matmul(
                                pss[m][:],
                                xts[k][:, m * 128 : (m + 1) * 128],
                                wts[k][:],
                                start=(k == 0),
                                stop=False,
                                skip_group_check=True,
                            )
                    for m in range(m_tiles):
                        for k in range(k_half, k_tiles):
                            nc.tensor.matmul(
                                pss[m][:],
                                xts[k][:, m * 128 : (m + 1) * 128],
                                wts[k][:],
                                start=False,
                                stop=(k == k_tiles - 1),
                                skip_group_check=True,
                            )
                        o_sb = opool.tile([128, n_tile], mybir.dt.float32, name="o_sb", tag="o_sb")
                        nc.vector.tensor_copy(o_sb[:], pss[m][:])
                        nc.scalar.dma_start(
                            out=bass.AP(
                                out, m * 128 * f + n * n_tile, [[f, 128], [1, n_tile]]
                            ),
                            in_=o_sb[:],
                        )
                else:
                    # m-outer / k-inner: staggered psum completion overlaps
                    # eviction + output DMA with compute.
                    for m in range(m_tiles):
                        ps = ppool.tile([128, n_tile], mybir.dt.float32, name="ps", tag="ps")
                        for k in range(k_tiles):
                            nc.tensor.matmul(
                                ps[:],
                                xts[k][:, m * 128 : (m + 1) * 128],
                                wts[k][:],
                                start=(k == 0),
                                stop=(k == k_tiles - 1),
                            )
                        o_sb = opool.tile([128, n_tile], mybir.dt.float32, name="o_sb", tag="o_sb")
                        nc.vector.tensor_copy(o_sb[:], ps[:])
                        nc.scalar.dma_start(
                            out=bass.AP(
                                out, m * 128 * f + n * n_tile, [[f, 128], [1, n_tile]]
                            ),
                            in_=o_sb[:],
                        )
                if n + 1 < n_tiles:
                    wts = nxt
    nc.compile()
    return nc


def _get_nc(mode):
    if mode not in _NC_CACHE:
        _NC_CACHE[mode] = _build_nc(mode)
    return _NC_CACHE[mode]


def _soft_topk_scale(alpha):
    a = alpha.astype(np.float64)
    e = np.exp(a - a.max())
    return np.clip(KTOPK * (e / e.sum()), 0.0, 1.0).astype(np.float32)


def kernel(x, V, alpha):
    global _LAST_RESULTS
    from concourse.bass_utils import run_bass_kernel_spmd

    x = np.asarray(x, dtype=np.float32)
    V = np.asarray(V, dtype=np.float32)
    alpha = np.asarray(alpha, dtype=np.float32)

    a_topk = _soft_topk_scale(alpha)
    VsT = np.ascontiguousarray((V * a_topk[:, None]).T)  # [c, p]
    W2 = np.concatenate([VsT, VsT], axis=1)  # [F, 2F]
    xT = np.ascontiguousarray(x.T)  # [F, B]

    mode = _MODE
    if mode == "bf16":
        W2 = W2.astype(ml_dtypes.bfloat16)
        xT = xT.astype(ml_dtypes.bfloat16)

    nc = _get_nc(mode)
    in_maps = [
        {"xt": np.ascontiguousarray(xT[:, i * BS : (i + 1) * BS]), "w2": W2}
        for i in range(NCORES)
    ]
    kwargs = {}
    if os.environ.get("GTOPK_TRACE"):
        kwargs["trace"] = True
    res = run_bass_kernel_spmd(nc, in_maps, core_ids=list(range(NCORES)), **kwargs)
    _LAST_RESULTS = res
    return np.concatenate([r["out"] for r in res.results], axis=0)


# revision 9
# speedup vs baseline: 1.2212x; 1.0295x over previous
"""Trainium2 Bass kernel for CustomFullyConnectedLayerGoogleTopK.

Math (from the reference, with IN_F == OUT_F == TOTAL_PERM == DIAG_LEN == 4096):
    a_topk = clip(K * softmax(alpha), 0, 1)                    # K = 3687
    Vs     = V * a_topk[:, None]                               # [4096, 4096]
    W[r,c] = Vs[(r - c) % 4096, c]   (scatter has no collisions)
    out    = x @ W.T                                           # [8192, 4096]

Device strategy: data-parallel over batch (8 cores x 1024 rows). The weight
W.T[c, r] = VsT[c, (r - c) % 4096] where VsT = Vs.T. Storing the doubled
array W2 = concat(VsT, VsT, axis=1) [4096, 8192] makes every [128, ntile]
tile of W.T a single affine access pattern: element (p, j) of the tile for
(k, n) lives at W2 linear offset (4096 + n*ntile) + p*8191 + k*128*8191 + j.
So the whole matmul streams with plain DMAs - no gather, no transpose.

Each core: out_slice[1024, 4096] = xT_slice.T @ W.T via PE matmuls,
lhsT = xT tile [128c, 128b] (stationary), rhs = W.T tile [128c, ntile r].
"""

import os

import numpy as np
import ml_dtypes

B = 8192  # batch
F = 4096  # in_features == out_features == total_perm == diag_len
NCORES = 8
BS = B // NCORES  # batch rows per core
KTOPK = 3687  # ceil((1 - 0.1) * F * F / F)

# "bf16" (full-rate PE, ~1.6e-3 rel err) or "fp32r" (fp32 storage,
# reduced-precision multiply).
_MODE = os.environ.get("GTOPK_MODE", "bf16")

_NC_CACHE = {}
_LAST_RESULTS = None  # stashed BassKernelResults for test harness introspection


def _build_nc(mode, f=F, bs=BS):
    import concourse.bass as bass
    import concourse.tile as tile
    from concourse import bacc, mybir

    if mode == "bf16":
        in_dt = mybir.dt.bfloat16
        n_tile = 512
    elif mode == "fp32r":
        in_dt = mybir.dt.float32r
        n_tile = 256
    else:
        raise ValueError(mode)

    k_tiles = f // 128
    m_tiles = bs // 128
    n_tiles = f // n_tile
    w2w = 2 * f  # doubled width

    nc = bacc.Bacc(None, target_bir_lowering=False, debug=False)
    xt = nc.dram_tensor("xt", [f, bs], in_dt, kind="ExternalInput")
    w2 = nc.dram_tensor("w2", [f, w2w], in_dt, kind="ExternalInput")
    out = nc.dram_tensor("out", [bs, f], mybir.dt.float32, kind="ExternalOutput")

    def xt_src(k):  # [128, bs] tile k of x.T slice
        return bass.AP(xt, k * 128 * bs, [[bs, 128], [1, bs]])

    def wt_src(n, k):  # staircase [128, n_tile] tile of W.T
        return bass.AP(
            w2, f + n * n_tile + k * 128 * (w2w - 1), [[w2w - 1, 128], [1, n_tile]]
        )

    with tile.TileContext(nc) as tc:
        with (
            tc.tile_pool(name="xpool", bufs=k_tiles) as xpool,
            tc.tile_pool(name="wpool", bufs=2 * k_tiles) as wpool,
            tc.tile_pool(name="opool", bufs=4) as opool,
            tc.tile_pool(name="ppool", bufs=8, space="PSUM") as ppool,
        ):
            # x.T slice cached in SBUF as 32 separate [128, bs] tiles so the
            # scheduler can start matmuls as soon as individual tiles land.
            # Interleave x/w DMAs for n=0 so (xt[k], wt[0,k]) pairs arrive
            # together.
            xts = []
            wt0 = []
            for k in range(k_tiles):
                xk = xpool.tile([128, bs], in_dt, name=f"xt{k}", tag="xt")
                nc.sync.dma_start(out=xk[:], in_=xt_src(k))
                wk = wpool.tile([128, n_tile], in_dt, name=f"wt0_{k}", tag="wt")
                nc.sync.dma_start(out=wk[:], in_=wt_src(0, k))
                xts.append(xk)
                wt0.append(wk)

            wts = wt0
            for n in range(n_tiles):
                # prefetch next n's weight tiles
                if n + 1 < n_tiles:
                    nxt = []
                    for k in range(k_tiles):
                        wk = wpool.tile(
                            [128, n_tile], in_dt, name=f"wt{n + 1}_{k}", tag="wt"
                        )
                        nc.sync.dma_start(out=wk[:], in_=wt_src(n + 1, k))
                        nxt.append(wk)
                if n == 0:
                    # Ramp phase: k-outer / m-inner over the first half of k
                    # so each arriving (xt[k], wt[k]) pair immediately feeds
                    # m_tiles matmuls (PE starts as soon as the first pair
                    # lands). Then finish per-m (k-inner) so the 8 psum banks
                    # complete staggered and evictions overlap compute.
                    k_half = k_tiles // 2
                    pss = [
                        ppool.tile([128, n_tile], mybir.dt.float32, name=f"ps{m}", tag="ps")
                        for m in range(m_tiles)
                    ]
                    for k in range(k_half):
                        for m in range(m_tiles):
                            nc.tensor.matmul(
                                pss[m][:],
                                xts[k][:, m * 128 : (m + 1) * 128],
                                wts[k][:],
                                start=(k == 0),
                                stop=False,
                                skip_group_check=True,
                            )
                    for m in range(m_tiles):
                        for k in range(k_half, k_tiles):
                            nc.tensor.matmul(
                                pss[m][:],
                                xts[k][:, m * 128 : (m + 1) * 128],
                                wts[k][:],
                                start=False,
                                stop=(k == k_tiles - 1),
                                skip_group_check=True,
                            )
                        o_sb = opool.tile([128, n_tile], mybir.dt.float32, name="o_sb", tag="o_sb")
                        nc.vector.tensor_copy(o_sb[:], pss[m][:])
                        nc.scalar.dma_start(
                            out=bass.AP(
                                out, m * 128 * f + n * n_tile, [[f, 128], [1, n_tile]]
                            ),
                            in_=o_sb[:],
                        )
                else:
                    # m-outer / k-inner: staggered psum completion overlaps
                    # eviction + output DMA with compute.
                    for m in range(m_tiles):
                        ps = ppool.tile([128, n_tile], mybir.dt.float32, name="ps", tag="ps")
                        for k in range(k_tiles):
                            nc.tensor.matmul(
                                ps[:],
                                xts[k][:, m * 128 : (m + 1) * 128],
                                wts[k][:],
                                start=(k == 0),
                                stop=(k == k_tiles - 1),
                            )
                        o_sb = opool.tile([128, n_tile], mybir.dt.float32, name="o_sb", tag="o_sb")
                        nc.vector.tensor_copy(o_sb[:], ps[:])
                        nc.scalar.dma_start(
                            out=bass.AP(
                                out, m * 128 * f + n * n_tile, [[f, 128], [1, n_tile]]
                            ),
                            in_=o_sb[:],
                        )
                if n + 1 < n_tiles:
                    wts = nxt
    nc.compile()
    return nc


def _get_nc(mode):
    if mode not in _NC_CACHE:
        _NC_CACHE[mode] = _build_nc(mode)
    return _NC_CACHE[mode]


def _soft_topk_scale(alpha):
    a = alpha.astype(np.float64)
    e = np.exp(a - a.max())
    return np.clip(KTOPK * (e / e.sum()), 0.0, 1.0).astype(np.float32)


def kernel(x, V, alpha):
    global _LAST_RESULTS
    from concourse.bass_utils import run_bass_kernel_spmd

    x = np.asarray(x, dtype=np.float32)
    V = np.asarray(V, dtype=np.float32)
    alpha = np.asarray(alpha, dtype=np.float32)

    a_topk = _soft_topk_scale(alpha)
    VsT = np.ascontiguousarray((V * a_topk[:, None]).T)  # [c, p]
    W2 = np.concatenate([VsT, VsT], axis=1)  # [F, 2F]
    xT = np.ascontiguousarray(x.T)  # [F, B]

    mode = _MODE
    if mode == "bf16":
        W2 = W2.astype(ml_dtypes.bfloat16)
        xT = xT.astype(ml_dtypes.bfloat16)

    nc = _get_nc(mode)
    in_maps = [
        {"xt": np.ascontiguousarray(xT[:, i * BS : (i + 1) * BS]), "w2": W2}
        for i in range(NCORES)
    ]
    kwargs = {}
    if os.environ.get("GTOPK_TRACE"):
        kwargs["trace"] = True
    res = run_bass_kernel_spmd(nc, in_maps, core_ids=list(range(NCORES)), **kwargs)
    _LAST_RESULTS = res
    return np.concatenate([r["out"] for r in res.results], axis=0)
